# revision 3
# baseline (speedup 1.0000x reference)
"""Prefix-mean along sequence dim (cumsum(x, axis=1) / (1..S)) on 8 TRN2 cores.

Strategy: pure data parallel over the batch dim (2 batches per core).
Per (batch, 127-row seq chunk) a single fp32 matmul against a fused
triangular matrix computes the chunk-local cumsum AND adds the running
carry: the carry rides in partition 0 of the rhs tile (column 0 of the
lhsT is all-ones so the new carry emerges lane-aligned in PSUM partition
0; the k==0 row broadcasts the incoming carry to every output row).
ScalarE/VectorE evict PSUM with a per-partition 1/(s+1) scale; the DMA
store skips the carry partition.
"""

import numpy as np

import bass_rust
import concourse.bass as bass
import concourse.mybir as mybir
from concourse.tile import TileContext
from concourse.bass_utils import run_bass_kernel_spmd

B, S, D = 16, 8192, 256
N_CORES = 8
BPC = B // N_CORES  # batches per core
CH = 127            # seq rows per full chunk (partitions 1..127)
NFULL = S // CH     # 64 full chunks
CLAST = S - NFULL * CH  # 64 rows in the final partial chunk
NCHUNK = NFULL + 1

F32 = mybir.dt.float32


def split_multi_waits(nc):
    """Walrus in this container accepts at most ONE sync wait per
    instruction; Tile emits more (notably on the tail drain and on
    cross-engine join points).  Hoist the extras onto fresh single-wait
    NoOps inserted just before the offending instruction."""
    n_split = 0
    for f in nc.m.functions:
        for blk in f.blocks:
            insts = blk.instructions
            if not any(
                getattr(i, "sync_info", None)
                and i.sync_info.on_wait
                and len(i.sync_info.on_wait) > 1
                for i in insts
            ):
                continue
            new = []
            for inst in insts:
                si = getattr(inst, "sync_info", None)
                waits = list(si.on_wait) if (si and si.on_wait) else []
                if len(waits) > 1:
                    for w in waits[:-1]:
                        nop = mybir.InstNoOp(
                            name=nc.get_next_instruction_name(), ins=[], outs=[]
                        )
                        nop.engine = inst.engine
                        nop.sync_info = bass_rust.SyncInfo(on_wait=[w], on_update=[])
                        new.append(nop)
                        n_split += 1
                    si.on_wait = waits[-1:]
                new.append(inst)
            blk.instructions = new
    return n_split


def make_consts():
    k = np.arange(128)[:, None]
    m = np.arange(128)[None, :]
    # full chunks: carry row (k=0) feeds every output; x row k feeds outputs
    # m >= k; column 0 gathers everything -> new carry.
    L1 = ((k == 0) | ((k >= 1) & (k <= m)) | (m == 0)).astype(np.float32)
    # last (partial) chunk: only x rows 1..CLAST are real; rows above are
    # zeroed so stale SBUF data cannot leak in.
    Llast = ((k == 0) | ((k >= 1) & (k <= np.minimum(m, CLAST)))).astype(np.float32)
    Llast[:, 0] = (k[:, 0] <= CLAST).astype(np.float32)
    R = np.ones((128, NCHUNK), np.float32)
    p = np.arange(128)
    for i in range(NCHUNK):
        # output partition p >= 1 holds seq position s = i*CH + p - 1
        R[1:, i] = 1.0 / (i * CH + p[1:]).astype(np.float64)
    return L1, Llast, R


def _emit_body(nc, rpool, opool, ppool, L1t, LLt, Rt, x, y):
    AF = mybir.ActivationFunctionType
    rhs = {}
    for b in range(BPC):
        t = rpool.tile([128, D], F32, tag=f"rhs{b}")
        nc.gpsimd.memset(t[0:1, :], 0.0)  # zero carry for chunk 0
        nc.sync.dma_start(t[1 : 1 + CH, :], x[b, 0:CH, :])
        rhs[b] = t
    for i in range(NCHUNK):
        last = i == NCHUNK - 1
        nrows = CLAST if last else CH
        for b in range(BPC):
            cur = rhs[b]
            ps = ppool.tile([128, D], F32, tag=f"ps{b}")
            nc.tensor.matmul(
                ps[:, :], lhsT=(LLt if last else L1t)[:, :], rhs=cur[:, :],
                start=True, stop=True,
            )
            if not last:
                nxt = rpool.tile([128, D], F32, tag=f"rhs{b}")
                s0n = (i + 1) * CH
                nn = CLAST if i + 1 == NCHUNK - 1 else CH
                if i + 1 == NCHUNK - 1:
                    # zero the whole tile first; the partial load and carry
                    # copy then overwrite partitions [1:1+nn] and [0:1]
                    nc.gpsimd.memset(nxt[:, :], 0.0)
                nc.sync.dma_start(nxt[1 : 1 + nn, :], x[b, s0n : s0n + nn, :])
                # chain the carry into partition 0 of the next rhs tile
                nc.scalar.copy(nxt[0:1, :], ps[0:1, :])
                rhs[b] = nxt
            ot = opool.tile([128, D], F32, tag=f"out{b}")
            if (i + b) % 2 == 0:
                nc.scalar.activation(
                    ot[:, :], ps[:, :], AF.Copy, scale=Rt[:, i : i + 1]
                )
            else:
                nc.vector.tensor_scalar_mul(ot[:, :], ps[:, :], Rt[:, i : i + 1])
            nc.sync.dma_start(y[b, i * CH : i * CH + nrows, :], ot[1 : 1 + nrows, :])


def build_program(n_iters=1):
    nc = bass.Bass()
    x = nc.declare_dram_parameter("x", [BPC, S, D], F32, isOutput=False)
    l1 = nc.declare_dram_parameter("L1", [128, 128], F32, isOutput=False)
    ll = nc.declare_dram_parameter("LL", [128, 128], F32, isOutput=False)
    r = nc.declare_dram_parameter("R", [128, NCHUNK], F32, isOutput=False)
    y = nc.declare_dram_parameter("y", [BPC, S, D], F32, isOutput=True)
    with TileContext(nc) as tc:
        with (
            tc.tile_pool(name="const", bufs=1) as cpool,
            tc.tile_pool(name="rhs", bufs=4) as rpool,
            tc.tile_pool(name="outp", bufs=4) as opool,
            tc.tile_pool(name="psum", bufs=3, space="PSUM") as ppool,
        ):
            L1t = cpool.tile([128, 128], F32, tag="L1")
            nc.sync.dma_start(L1t[:, :], l1[:, :])
            LLt = cpool.tile([128, 128], F32, tag="LL")
            nc.sync.dma_start(LLt[:, :], ll[:, :])
            Rt = cpool.tile([128, NCHUNK], F32, tag="R")
            nc.sync.dma_start(Rt[:, :], r[:, :])
            for it in range(n_iters):
                if it:
                    tc.strict_bb_all_engine_barrier()
                _emit_body(nc, rpool, opool, ppool, L1t, LLt, Rt, x, y)
    split_multi_waits(nc)
    return nc


_nc_cache = {}


def get_program(n_iters=1):
    if n_iters not in _nc_cache:
        _nc_cache[n_iters] = build_program(n_iters)
    return _nc_cache[n_iters]


def make_in_maps(x):
    L1, LL, R = make_consts()
    return [
        {
            "x": np.ascontiguousarray(x[c * BPC : (c + 1) * BPC]),
            "L1": L1,
            "LL": LL,
            "R": R,
        }
        for c in range(N_CORES)
    ]


def run(x, n_iters=1):
    nc = get_program(n_iters)
    res = run_bass_kernel_spmd(nc, make_in_maps(x), list(range(N_CORES)))
    return np.concatenate([res.results[c]["y"] for c in range(N_CORES)], axis=0)


def kernel(inputs_embeds):
    x = np.asarray(inputs_embeds, dtype=np.float32)
    assert x.shape == (B, S, D), x.shape
    return run(x, n_iters=1)


if __name__ == "__main__":
    rng = np.random.default_rng(0)
    x = rng.standard_normal((B, S, D), dtype=np.float32)
    got = kernel(x)
    ref = np.cumsum(x, axis=1, dtype=np.float64) / np.arange(1, S + 1, dtype=np.float64)[None, :, None]
    ref = ref.astype(np.float32)
    err = np.linalg.norm(got - ref) / np.linalg.norm(ref)
    print("rel err:", err)
    print("absmax err:", np.abs(got - ref).max(), "ref absmax:", np.abs(ref).max())


# revision 10
# speedup vs baseline: 1.1902x; 1.1902x over previous
"""Prefix-mean along sequence dim (cumsum(x, axis=1) / (1..S)) on 8 TRN2 cores.

Per core (2 batches): two-level scan, fp32-exact.
  Layout: partition p = d2*64 + sc  (d2 = d-half, sc = seq chunk of 128 rows);
  each partition holds [j=128 seq rows, dof=128 d-cols] contiguously (64 KB).
  DMA in/out moves 512B-contiguous runs -> full HBM bandwidth.

  Phase 1: DVE tensor_tensor_scan per (batch, dof): chunk-local cumsum along
  j (in-place, strided free dim).  Chunk totals = the j=127 slice.
  Carry:   ONE fp32 matmul  C = Lstrict^T @ totals  (block-diag strict lower
  triangular over sc within each d2) -- matmul instructions cost ~35us on
  this deployment, so exactly one is used.
  Phase 2: DVE in-place (x + C_bcast) * R_bcast with zero-stride broadcast
  APs; ScalarE is avoided entirely (ACTIVATE ~80us/op here), weights load
  only once (Tile's per-matmul Ldweights are deduplicated post-hoc).
"""

import numpy as np

import bass_rust
import concourse.bass as bass
import concourse.mybir as mybir
from concourse.tile import TileContext
from concourse.bass_utils import run_bass_kernel_spmd

B, S, D = 16, 8192, 256
N_CORES = 8
BPC = B // N_CORES  # batches per core
SC = 64             # seq chunks per d-half  (partitions = 2*SC = 128)
J = S // SC         # 128 seq rows per chunk
DOF = D // 2        # 128 d-cols per d-half
FREE = J * DOF      # 16384 elements per partition per batch

F32 = mybir.dt.float32
AOT = mybir.AluOpType


def split_multi_waits(nc):
    """Walrus in this container accepts at most ONE sync wait per
    instruction; Tile emits more (notably on the tail drain and cross-engine
    join points).  Hoist extras onto single-wait NoOps inserted before the
    offending instruction."""
    n_split = 0
    for f in nc.m.functions:
        for blk in f.blocks:
            insts = blk.instructions
            if not any(
                getattr(i, "sync_info", None)
                and i.sync_info.on_wait
                and len(i.sync_info.on_wait) > 1
                for i in insts
            ):
                continue
            new = []
            for inst in insts:
                si = getattr(inst, "sync_info", None)
                waits = list(si.on_wait) if (si and si.on_wait) else []
                if len(waits) > 1:
                    for w in waits[:-1]:
                        nop = mybir.InstNoOp(
                            name=nc.get_next_instruction_name(), ins=[], outs=[]
                        )
                        nop.engine = inst.engine
                        nop.sync_info = bass_rust.SyncInfo(on_wait=[w], on_update=[])
                        new.append(nop)
                        n_split += 1
                    si.on_wait = waits[-1:]
                new.append(inst)
            blk.instructions = new
    return n_split


def ldweights_to_nops(nc):
    """Tile emits an Ldweights before every Matmult; each costs ~70us on this
    deployment.  Replace repeats with identical weight APs by NoOps (keeping
    their sync_info)."""
    n = 0
    for f in nc.m.functions:
        for blk in f.blocks:
            cur_sig = None
            new = []
            for inst in blk.instructions:
                if inst.opcode == "Ldweights":
                    sig = str(inst.ins[0])
                    if sig == cur_sig:
                        nop = mybir.InstNoOp(
                            name=nc.get_next_instruction_name(), ins=[], outs=[]
                        )
                        nop.engine = inst.engine
                        if inst.sync_info is not None:
                            nop.sync_info = inst.sync_info
                        new.append(nop)
                        n += 1
                        continue
                    cur_sig = sig
                new.append(inst)
            blk.instructions = new
    return n


def make_consts():
    # block-diag strict lower triangular: C[m=(d2,sc)] = sum_{sc'<sc} t[(d2,sc')]
    k = np.arange(128)
    m = np.arange(128)
    ke2, ksc = k // SC, k % SC
    md2, msc = m // SC, m % SC
    L = ((ke2[:, None] == md2[None, :]) & (ksc[:, None] < msc[None, :])).astype(
        np.float32
    )
    # R[p, j] = 1/(s+1) with s = (p%SC)*J + j
    p = np.arange(128)
    j = np.arange(J)
    R = (
        1.0 / ((p % SC)[:, None] * J + j[None, :] + 1.0).astype(np.float64)
    ).astype(np.float32)
    return L, R


def _x_ap(dram, b, d2):
    """DRAM-side AP for batch b, d-half d2, matching the on-chip layout:
    partitions sc, free (j, dof)."""
    return dram[b][:, d2 * DOF : (d2 + 1) * DOF].rearrange(
        "(sc j) dof -> sc j dof", sc=SC, j=J
    )


def _emit_body(nc, pool, ppool, Lt, Rt, tot_pool, x, y):
    ts = []
    for b in range(BPC):
        t = pool.tile([128, FREE], F32, tag=f"t{b}")
        t4 = t[:, :].rearrange("p (j dof) -> p j dof", j=J, dof=DOF)
        for d2 in range(2):
            src = _x_ap(x, b, d2)
            psl = slice(d2 * SC, (d2 + 1) * SC)
            for h in range(2):
                jsl = slice(h * J // 2, (h + 1) * J // 2)
                nc.sync.dma_start(t4[psl, jsl, :], src[:, jsl, :])
        ts.append(t)

    totals = tot_pool.tile([128, BPC * DOF], F32, tag="totals")
    for b in range(BPC):
        t = ts[b]
        t3 = t[:, :].rearrange("p (j dof) -> p dof j", j=J, dof=DOF)
        for dof in range(DOF):
            lane = t3[:, dof, :]
            nc.vector.tensor_tensor_scan(lane, lane, lane, 0.0, AOT.add, AOT.bypass)
        # chunk totals live in the last j slice (contiguous)
        nc.vector.tensor_copy(
            totals[:, b * DOF : (b + 1) * DOF], t[:, (J - 1) * DOF :]
        )

    ps = ppool.tile([128, BPC * DOF], F32, tag="psC")
    nc.tensor.matmul(ps[:, :], lhsT=Lt[:, :], rhs=totals[:, :], start=True, stop=True)
    Ct = tot_pool.tile([128, BPC * DOF], F32, tag="Ct")
    nc.vector.tensor_copy(Ct[:, :], ps[:, :])

    for b in range(BPC):
        t = ts[b]
        t3 = t[:, :].rearrange("p (j dof) -> p j dof", j=J, dof=DOF)
        cb = Ct[:, b * DOF : (b + 1) * DOF][:, None, :]
        for h in range(2):
            jsl = slice(h * J // 2, (h + 1) * J // 2)
            t3h = t3[:, jsl, :]
            nc.vector.tensor_tensor(
                out=t3h, in0=t3h, in1=cb.broadcast_to([128, J // 2, DOF]), op=AOT.add
            )
            rb = Rt[:, jsl][:, :, None].broadcast_to([128, J // 2, DOF])
            nc.vector.tensor_tensor(out=t3h, in0=t3h, in1=rb, op=AOT.mult)
            for d2 in range(2):
                dst = _x_ap(y, b, d2)
                psl = slice(d2 * SC, (d2 + 1) * SC)
                nc.sync.dma_start(dst[:, jsl, :], t3h[psl, :, :])


def build_program(n_iters=1):
    nc = bass.Bass()
    x = nc.declare_dram_parameter("x", [BPC, S, D], F32, isOutput=False)
    l = nc.declare_dram_parameter("L", [128, 128], F32, isOutput=False)
    r = nc.declare_dram_parameter("R", [128, J], F32, isOutput=False)
    y = nc.declare_dram_parameter("y", [BPC, S, D], F32, isOutput=True)
    with TileContext(nc) as tc:
        with (
            tc.tile_pool(name="const", bufs=1) as cpool,
            tc.tile_pool(name="data", bufs=1) as pool,
            tc.tile_pool(name="tot", bufs=2) as tot_pool,
            tc.tile_pool(name="psum", bufs=2, space="PSUM") as ppool,
        ):
            Lt = cpool.tile([128, 128], F32, tag="L")
            nc.sync.dma_start(Lt[:, :], l[:, :])
            Rt = cpool.tile([128, J], F32, tag="R")
            nc.sync.dma_start(Rt[:, :], r[:, :])
            for it in range(n_iters):
                if it:
                    tc.strict_bb_all_engine_barrier()
                _emit_body(nc, pool, ppool, Lt, Rt, tot_pool, x, y)
    split_multi_waits(nc)
    ldweights_to_nops(nc)
    return nc


_nc_cache = {}


def get_program(n_iters=1):
    if n_iters not in _nc_cache:
        _nc_cache[n_iters] = build_program(n_iters)
    return _nc_cache[n_iters]


def make_in_maps(x):
    L, R = make_consts()
    return [
        {"x": np.ascontiguousarray(x[c * BPC : (c + 1) * BPC]), "L": L, "R": R}
        for c in range(N_CORES)
    ]


def run(x, n_iters=1):
    nc = get_program(n_iters)
    res = run_bass_kernel_spmd(nc, make_in_maps(x), list(range(N_CORES)))
    return np.concatenate([res.results[c]["y"] for c in range(N_CORES)], axis=0)


def kernel(inputs_embeds):
    x = np.asarray(inputs_embeds, dtype=np.float32)
    assert x.shape == (B, S, D), x.shape
    return run(x, n_iters=1)


if __name__ == "__main__":
    rng = np.random.default_rng(0)
    x = rng.standard_normal((B, S, D), dtype=np.float32)
    got = kernel(x)
    ref = np.cumsum(x, axis=1, dtype=np.float64) / np.arange(1, S + 1, dtype=np.float64)[None, :, None]
    ref = ref.astype(np.float32)
    err = np.linalg.norm(got - ref) / np.linalg.norm(ref)
    print("rel err:", err)
    print("absmax err:", np.abs(got - ref).max(), "ref absmax:", np.abs(ref).max())
